# revision 16
# baseline (speedup 1.0000x reference)
"""TRN2 Bass kernel for nn_Encoder_27290222198965.

Reference computation (N=8, L=2048, H=1024):
    q = x@Wq.T+bq ; k = x@Wk.T+bk ; v = x@Wv.T+bv
    d[n,l] = sum_h q*k                       (diagonal "attention" scores)
    att = softmax(diag-embed(d), axis=2) ->  colsum[n,l] = S[n] + (e-1)/(L-1+e),
        e = exp(d[n,l]), S[n] = sum_l 1/(L-1+exp(d[n,l]))
    out = (colsum[:, :, None] * v) @ Wo.T + bo

Algebraic refactor (validated ~3e-3 rel err with bf16 operands):
    d[n,l] = rowsum(x o y') + c0,  y' = x @ M^T + u,
        M = Wq^T Wk, u = Wk^T bq + Wq^T bk, c0 = bq.bk
    colsum = (S+1) - 2048*r,  r = 1/(2047+exp(d)),  S = sum_l r
        (uses e*r = 1 - 2047*r)
    out    = colsum o (x @ Wc^T + bc) + bo,  Wc = Wo@Wv, bc = Wo@bv
so only TWO HxH projections run on hardware (y' and z) instead of four.

v3 design (vs 149us fp32r baseline; v2 @ 134us):
  - all matmul operands bf16: enables compiler fast-weight-load (disabled
    for fp32), halves DMA. PSUM stays fp32. MM pitch hits the 216ns floor.
  - x resident in SBUF (4MB bf16), loaded once; phase 2 re-uses it.
  - batched DMA: lb0's x as 4 pair tiles, lb1-3 as one 1MB DMA each, mt
    grouped (1+2+4), wct as one 2MB DMA. Few DMA instructions -> no
    issue-queue/semaphore-recycle serialization in the cold start.
  - elementwise fused via scalar_tensor_tensor reading PSUM directly;
    sigmoid emits per-block S partials via accum_out.
  - out DMAs alternate sync/scalar; the last l-block's go to the idle
    gpsimd queue; final group split 2x256 to shorten the tail.

Sharding: data-parallel over N -- core n handles batch n.
"""

import numpy as np
import ml_dtypes

import concourse.bass as bass  # noqa: F401  (registers engines on Bacc)
import concourse.tile as tile
from concourse import bacc, mybir
from concourse.bass_utils import run_bass_kernel_spmd

dt = mybir.dt
AF = mybir.ActivationFunctionType
ALU = mybir.AluOpType

N, L, H = 8, 2048, 1024
P = 128            # SBUF partitions
LB = 512           # l-block (moving free dim of every matmul)
NH = H // P        # 8 h-blocks
NL = L // LB       # 4 l-blocks
N_CORES = 8
NC = NH + 1 + NH + NH   # cpack cols: ub, c0b, bcb, bob

_CACHE = {}


def _build():
    nc = bacc.Bacc("TRN2", target_bir_lowering=False, debug=False,
                   num_devices=N_CORES)

    xT_d = nc.dram_tensor("xT", [H, L], dt.bfloat16, kind="ExternalInput").ap()
    MT_d = nc.dram_tensor("MT", [P, NH, NH * P], dt.bfloat16,
                          kind="ExternalInput").ap()
    WcT_d = nc.dram_tensor("WcT", [P, NH, NH * P], dt.bfloat16,
                           kind="ExternalInput").ap()
    cp_d = nc.dram_tensor("cpack", [P, NC], dt.float32, kind="ExternalInput").ap()
    ones_d = nc.dram_tensor("ones", [P, P], dt.bfloat16,
                            kind="ExternalInput").ap()
    out_d = nc.dram_tensor("outT", [H, L], dt.float32, kind="ExternalOutput").ap()

    xT3 = xT_d.rearrange("(j p) l -> p j l", p=P)    # [128, 8, 2048]

    with tile.TileContext(nc) as tc:
        with (
            tc.tile_pool(name="resident", bufs=1) as rp,
            tc.tile_pool(name="weights", bufs=1) as wtp,
            tc.tile_pool(name="work", bufs=3) as wp,
            tc.tile_pool(name="mmpsum", bufs=6, space="PSUM") as yp,
            tc.tile_pool(name="dpsum", bufs=1, space="PSUM") as dp,
        ):
            t_s = rp.tile([P, L], dt.float32)
            cs = rp.tile([P, L], dt.float32)

            # ---- cold-start DMA emission: few, batched, priority order ----
            # first MM group needs mt0[hb0] + x(lb0, hb0) asap
            mt0c0 = wtp.tile([P, P], dt.bfloat16, tag="mt0c0")
            nc.scalar.dma_start(mt0c0[:], MT_d[:, 0, 0:P])
            xp0 = []
            for i in range(4):      # lb0 x as pairs of h-blocks
                t = wtp.tile([P, 2, LB], dt.bfloat16, name=f"xp0_{i}",
                             tag=f"xp0_{i}")
                eng = [nc.sync, nc.scalar, nc.sync, nc.scalar][i]
                eng.dma_start(t[:], xT3[:, 2 * i:2 * i + 2, 0:LB])
                xp0.append(t)
                if i == 0:
                    cp = rp.tile([P, NC], dt.float32)
                    nc.sync.dma_start(cp[:], cp_d[:])
                    mt0c1 = wtp.tile([P, 3 * P], dt.bfloat16, tag="mt0c1")
                    nc.gpsimd.dma_start(mt0c1[:], MT_d[:, 0, P:4 * P])
                    mt0c2 = wtp.tile([P, 4 * P], dt.bfloat16, tag="mt0c2")
                    nc.gpsimd.dma_start(mt0c2[:], MT_d[:, 0, 4 * P:8 * P])
            mt1 = wtp.tile([P, NH * P], dt.bfloat16, tag="mt1")
            nc.gpsimd.dma_start(mt1[:], MT_d[:, 1, :])
            xl = {}
            for lb, eng in ((1, nc.sync), (2, nc.scalar), (3, nc.sync)):
                t = wtp.tile([P, NH, LB], dt.bfloat16, name=f"xl{lb}",
                             tag=f"xl{lb}")
                eng.dma_start(t[:], xT3[:, :, lb * LB:(lb + 1) * LB])
                xl[lb] = t
            mt23 = wtp.tile([P, 2, NH * P], dt.bfloat16, tag="mt23")
            nc.gpsimd.dma_start(mt23[:], MT_d[:, 2:4, :])
            ones = rp.tile([P, P], dt.bfloat16)
            nc.gpsimd.dma_start(ones[:], ones_d[:])
            mt47 = wtp.tile([P, 4, NH * P], dt.bfloat16, tag="mt47")
            nc.gpsimd.dma_start(mt47[:], MT_d[:, 4:8, :])
            wct8 = wtp.tile([P, NH, NH * P], dt.bfloat16, tag="wct8")
            nc.gpsimd.dma_start(wct8[:], WcT_d[:])

            ub = cp[:, :NH]
            c0b = cp[:, NH:NH + 1]
            bcb = cp[:, NH + 1:NH + 1 + NH]
            bob = cp[:, NH + 1 + NH:]

            def mt_ap(ob, hb):
                cps = slice(hb * P, (hb + 1) * P)
                if ob == 0:
                    if hb == 0:
                        return mt0c0[:]
                    if hb < 4:
                        return mt0c1[:, (hb - 1) * P:hb * P]
                    return mt0c2[:, (hb - 4) * P:(hb - 3) * P]
                if ob == 1:
                    return mt1[:, cps]
                if ob < 4:
                    return mt23[:, ob - 2, cps]
                return mt47[:, ob - 4, cps]

            def x_ap(lb, hb):
                if lb == 0:
                    return xp0[hb // 2][:, hb % 2, :]
                return xl[lb][:, hb, :]

            sp = [rp.tile([P, 1], dt.float32, name=f"sp{i}", tag=f"sp{i}")
                  for i in range(NL)]

            # d-matmul bookkeeping: delay each block's last rowsum-MM into the
            # next MM group so the PE never waits on the DVE prod chain.
            state = {"pending": None}

            def flush_pending():
                if state["pending"] is None:
                    return
                pd_t, prod_t, lb = state["pending"]
                state["pending"] = None
                nc.tensor.matmul(pd_t[:], ones[:], prod_t[:],
                                 start=True, stop=True)
                # t = sigmoid(-d - c0 + ln(L-1)); r = t/(L-1)
                # accum_out gives S-partial = sum_l t over this block
                ls = slice(lb * LB, (lb + 1) * LB)
                nc.scalar.activation(t_s[:, ls], pd_t[:], AF.Sigmoid,
                                     bias=c0b[:, 0:1], scale=-1.0,
                                     accum_out=sp[lb][:])

            # ================= phase 1: y' ; d ; r ==================
            for lb in range(NL):
                pd = dp.tile([P, LB], dt.float32)
                acc = None
                for ob in range(NH):
                    py = yp.tile([P, LB], dt.float32, tag="mm")
                    for hb in range(NH):
                        nc.tensor.matmul(
                            py[:], mt_ap(ob, hb), x_ap(lb, hb),
                            start=(hb == 0), stop=(hb == NH - 1))
                    if ob == 1:
                        flush_pending()
                    # prod = (psum + u[ob]) * x   -- one fused DVE op
                    prod = wp.tile([P, LB], dt.float32r, tag="prod")
                    nc.vector.scalar_tensor_tensor(
                        prod[:], py[:], ub[:, ob:ob + 1], x_ap(lb, ob),
                        op0=ALU.add, op1=ALU.mult)
                    if acc is None:
                        acc = prod
                    else:
                        # last add emits bf16 so the rowsum MM stays in the
                        # bf16 pipeline (no fp32<->bf16 PE mode switch)
                        last = (ob == NH - 1)
                        nacc = wp.tile(
                            [P, LB],
                            dt.bfloat16 if last else dt.float32r,
                            name="naccb" if last else "nacc",
                            tag="paccb" if last else "pacc")
                        nc.vector.tensor_tensor(nacc[:], acc[:], prod[:],
                                                op=ALU.add)
                        acc = nacc
                state["pending"] = (pd, acc, lb)

            # ================= phase 2: z ; out ==================
            for lb in range(NL):
                for ob in range(NH):
                    last_grp = (lb == NL - 1 and ob == NH - 1)
                    nmm = 2 if last_grp else 1
                    mw = LB // nmm
                    pzs = []
                    for ck in range(nmm):
                        pz = yp.tile([P, mw], dt.float32, tag="mm")
                        for hb in range(NH):
                            nc.tensor.matmul(
                                pz[:], wct8[:, ob, hb * P:(hb + 1) * P],
                                x_ap(lb, hb)[:, ck * mw:(ck + 1) * mw],
                                start=(hb == 0), stop=(hb == NH - 1))
                        pzs.append(pz)
                    if lb == 0 and ob == 0:
                        flush_pending()   # last d-MM + sigmoid of phase 1
                        # colsum = (1 + sum(t)/(L-1)) - (L/(L-1))*t
                        s01 = rp.tile([P, 1], dt.float32)
                        nc.vector.tensor_tensor(s01[:], sp[0][:], sp[1][:],
                                                op=ALU.add)
                        s23 = rp.tile([P, 1], dt.float32)
                        nc.vector.tensor_tensor(s23[:], sp[2][:], sp[3][:],
                                                op=ALU.add)
                        s_all = rp.tile([P, 1], dt.float32)
                        nc.vector.tensor_tensor(s_all[:], s01[:], s23[:],
                                                op=ALU.add)
                        S1_t = rp.tile([P, 1], dt.float32)
                        nc.vector.tensor_scalar(
                            S1_t[:], s_all[:], 1.0 / (L - 1), 1.0,
                            op0=ALU.mult, op1=ALU.add)
                        nc.vector.tensor_scalar(
                            cs[:], t_s[:], -float(L) / (L - 1), S1_t[:],
                            op0=ALU.mult, op1=ALU.add)
                    for ck in range(nmm):
                        lo = lb * LB + ck * mw
                        lsc = slice(lo, lo + mw)
                        # z*colsum = (psum + bc[ob]) * cs  -- fused DVE op
                        zc = wp.tile([P, mw], dt.float32, tag="zc")
                        nc.vector.scalar_tensor_tensor(
                            zc[:], pzs[ck][:], bcb[:, ob:ob + 1], cs[:, lsc],
                            op0=ALU.add, op1=ALU.mult)
                        ot = wp.tile([P, mw], dt.float32, tag="ot")
                        nc.vector.tensor_scalar_add(
                            ot[:], zc[:], bob[:, ob:ob + 1])
                        if lb == NL - 1:
                            eng = nc.gpsimd
                        else:
                            eng = nc.scalar if (ob + ck) % 2 else nc.sync
                        eng.dma_start(out_d[ob * P:(ob + 1) * P, lsc], ot[:])

    nc.compile()
    return nc


def _get_nc():
    if "nc" not in _CACHE:
        _CACHE["nc"] = _build()
    return _CACHE["nc"]


def _prep_inputs(x, Wq, bq, Wk, bk, Wv, bv, Wo, bo):
    """Host-side precompute (fp64 for the fused weights) + per-core sharding."""
    f8 = np.float64
    bf = ml_dtypes.bfloat16
    M = (Wq.astype(f8).T @ Wk.astype(f8)).astype(np.float32)
    u = (Wk.astype(f8).T @ bq.astype(f8)
         + Wq.astype(f8).T @ bk.astype(f8)).astype(np.float32)
    c0 = np.float32(bq.astype(f8) @ bk.astype(f8))
    Wc = (Wo.astype(f8) @ Wv.astype(f8)).astype(np.float32)
    bc = (Wo.astype(f8) @ bv.astype(f8)).astype(np.float32)

    def _pack(WT):  # [H,H] (hin, hout) -> [P(hin%P), NH(ob), NH*P]
        t = WT.reshape(NH, P, NH, P)          # [hb, p, ob, c]
        return np.ascontiguousarray(
            t.transpose(1, 2, 0, 3).reshape(P, NH, NH * P).astype(bf))

    MT = _pack(M.T)
    WcT = _pack(Wc.T)
    ub = u.reshape(NH, P).T
    bcb = bc.reshape(NH, P).T
    bob = bo.astype(np.float32).reshape(NH, P).T
    c0b = np.full((P, 1), np.log(L - 1.0) - np.float64(c0), np.float32)
    cpack = np.ascontiguousarray(
        np.concatenate([ub, c0b, bcb, bob], axis=1).astype(np.float32))
    ones = np.ones((P, P), bf)

    shared = dict(MT=MT, WcT=WcT, cpack=cpack, ones=ones)
    in_maps = []
    for n in range(N_CORES):
        xT = np.ascontiguousarray(x[n].T.astype(bf))
        in_maps.append(dict(xT=xT, **shared))
    return in_maps


def kernel(x, Wq, bq, Wk, bk, Wv, bv, Wo, bo, _trace=False, _trace_kwargs=None):
    x, Wq, bq, Wk, bk, Wv, bv, Wo, bo = (
        np.asarray(a) for a in (x, Wq, bq, Wk, bk, Wv, bv, Wo, bo))
    nc = _get_nc()
    in_maps = _prep_inputs(x, Wq, bq, Wk, bk, Wv, bv, Wo, bo)
    res = run_bass_kernel_spmd(nc, in_maps, list(range(N_CORES)),
                               trace=_trace, **(_trace_kwargs or {}))
    out = np.empty((N, L, H), np.float32)
    for n in range(N_CORES):
        out[n] = res.results[n]["outT"].T
    if _trace:
        kernel.last_result = res
    return out


# revision 21
# speedup vs baseline: 1.0079x; 1.0079x over previous
"""TRN2 Bass kernel for nn_Encoder_27290222198965.

Reference computation (N=8, L=2048, H=1024):
    q = x@Wq.T+bq ; k = x@Wk.T+bk ; v = x@Wv.T+bv
    d[n,l] = sum_h q*k                       (diagonal "attention" scores)
    att = softmax(diag-embed(d), axis=2) ->  colsum[n,l] = S[n] + (e-1)/(L-1+e),
        e = exp(d[n,l]), S[n] = sum_l 1/(L-1+exp(d[n,l]))
    out = (colsum[:, :, None] * v) @ Wo.T + bo

Algebraic refactor (validated ~3e-3 rel err with bf16 operands):
    d[n,l] = rowsum(x o y') + c0,  y' = x @ M^T + u,
        M = Wq^T Wk, u = Wk^T bq + Wq^T bk, c0 = bq.bk
    colsum = (S+1) - 2048*r,  r = 1/(2047+exp(d)),  S = sum_l r
        (uses e*r = 1 - 2047*r)
    out    = colsum o (x @ Wc^T + bc) + bo,  Wc = Wo@Wv, bc = Wo@bv
so only TWO HxH projections run on hardware (y' and z) instead of four.

v3 design (vs 149us fp32r baseline; v2 @ 134us):
  - all matmul operands bf16: enables compiler fast-weight-load (disabled
    for fp32), halves DMA. PSUM stays fp32. MM pitch hits the 216ns floor.
  - x resident in SBUF (4MB bf16), loaded once; phase 2 re-uses it.
  - batched DMA: lb0's x as 4 pair tiles, lb1-3 as one 1MB DMA each, mt
    grouped (1+2+4), wct as one 2MB DMA. Few DMA instructions -> no
    issue-queue/semaphore-recycle serialization in the cold start.
  - elementwise fused via scalar_tensor_tensor reading PSUM directly;
    sigmoid emits per-block S partials via accum_out.
  - out DMAs alternate sync/scalar; the last l-block's go to the idle
    gpsimd queue; final group split 2x256 to shorten the tail.

Sharding: data-parallel over N -- core n handles batch n.
"""

import numpy as np
import ml_dtypes

import concourse.bass as bass  # noqa: F401  (registers engines on Bacc)
import concourse.tile as tile
from concourse import bacc, mybir
from concourse.bass_utils import run_bass_kernel_spmd

dt = mybir.dt
AF = mybir.ActivationFunctionType
ALU = mybir.AluOpType

N, L, H = 8, 2048, 1024
P = 128            # SBUF partitions
LB = 512           # l-block (moving free dim of every matmul)
NH = H // P        # 8 h-blocks
NL = L // LB       # 4 l-blocks
N_CORES = 8
NC = NH + 1 + NH + NH   # cpack cols: ub, c0b, bcb, bob

_CACHE = {}


def _build():
    nc = bacc.Bacc("TRN2", target_bir_lowering=False, debug=False,
                   num_devices=N_CORES)

    xT_d = nc.dram_tensor("xT", [H, L], dt.bfloat16, kind="ExternalInput").ap()
    MT_d = nc.dram_tensor("MT", [P, NH, NH * P], dt.bfloat16,
                          kind="ExternalInput").ap()
    WcT_d = nc.dram_tensor("WcT", [P, NH, NH * P], dt.bfloat16,
                           kind="ExternalInput").ap()
    cp_d = nc.dram_tensor("cpack", [P, NC], dt.float32, kind="ExternalInput").ap()
    ones_d = nc.dram_tensor("ones", [P, P], dt.bfloat16,
                            kind="ExternalInput").ap()
    out_d = nc.dram_tensor("outT", [H, L], dt.float32, kind="ExternalOutput").ap()

    xT3 = xT_d.rearrange("(j p) l -> p j l", p=P)    # [128, 8, 2048]

    with tile.TileContext(nc) as tc:
        with (
            tc.tile_pool(name="resident", bufs=1) as rp,
            tc.tile_pool(name="weights", bufs=1) as wtp,
            tc.tile_pool(name="work", bufs=3) as wp,
            tc.tile_pool(name="mmpsum", bufs=6, space="PSUM") as yp,
            tc.tile_pool(name="dpsum", bufs=1, space="PSUM") as dp,
        ):
            t_s = rp.tile([P, L], dt.float32)
            cs = rp.tile([P, L], dt.float32)

            # ---- cold-start DMA emission: few, batched, priority order ----
            # first MM group needs mt0[hb0] + x(lb0, hb0) asap
            mt0c0 = wtp.tile([P, P], dt.bfloat16, tag="mt0c0")
            nc.scalar.dma_start(mt0c0[:], MT_d[:, 0, 0:P])
            xp0 = []
            for i in range(4):      # lb0 x as pairs of h-blocks
                t = wtp.tile([P, 2, LB], dt.bfloat16, name=f"xp0_{i}",
                             tag=f"xp0_{i}")
                eng = [nc.sync, nc.scalar, nc.sync, nc.scalar][i]
                eng.dma_start(t[:], xT3[:, 2 * i:2 * i + 2, 0:LB])
                xp0.append(t)
                if i == 0:
                    cp = rp.tile([P, NC], dt.float32)
                    nc.sync.dma_start(cp[:], cp_d[:])
                    mt0c1 = wtp.tile([P, 3 * P], dt.bfloat16, tag="mt0c1")
                    nc.gpsimd.dma_start(mt0c1[:], MT_d[:, 0, P:4 * P])
                    mt0c2 = wtp.tile([P, 4 * P], dt.bfloat16, tag="mt0c2")
                    nc.gpsimd.dma_start(mt0c2[:], MT_d[:, 0, 4 * P:8 * P])
            mt1 = wtp.tile([P, NH * P], dt.bfloat16, tag="mt1")
            nc.gpsimd.dma_start(mt1[:], MT_d[:, 1, :])
            # x for lb1 split in hb-halves for finer arrival pacing
            x1h = []
            for i in range(2):
                t = wtp.tile([P, 4, LB], dt.bfloat16, name=f"x1h{i}",
                             tag=f"x1h{i}")
                nc.sync.dma_start(t[:], xT3[:, 4 * i:4 * i + 4, LB:2 * LB])
                x1h.append(t)
            mt23 = wtp.tile([P, 2, NH * P], dt.bfloat16, tag="mt23")
            nc.gpsimd.dma_start(mt23[:], MT_d[:, 2:4, :])
            mt45 = wtp.tile([P, 2, NH * P], dt.bfloat16, tag="mt45")
            nc.gpsimd.dma_start(mt45[:], MT_d[:, 4:6, :])
            mt67 = wtp.tile([P, 2, NH * P], dt.bfloat16, tag="mt67")
            nc.gpsimd.dma_start(mt67[:], MT_d[:, 6:8, :])
            # x(lb2,lb3) + ones + phase-2 weights are issued on gpsimd
            # *behind a compute op that waits on lb0-ob4 data* so their
            # transfers can't steal cold-start DMA bandwidth (see ob==4
            # in phase 1 below)
            xl = {}
            for lb in (2, 3):
                xl[lb] = wtp.tile([P, NH, LB], dt.bfloat16, name=f"xl{lb}",
                                  tag=f"xl{lb}")
            ones = rp.tile([P, P], dt.bfloat16)
            wct8 = wtp.tile([P, NH, NH * P], dt.bfloat16, tag="wct8")

            ub = cp[:, :NH]
            c0b = cp[:, NH:NH + 1]
            bcb = cp[:, NH + 1:NH + 1 + NH]
            bob = cp[:, NH + 1 + NH:]

            def mt_ap(ob, hb):
                cps = slice(hb * P, (hb + 1) * P)
                if ob == 0:
                    if hb == 0:
                        return mt0c0[:]
                    if hb < 4:
                        return mt0c1[:, (hb - 1) * P:hb * P]
                    return mt0c2[:, (hb - 4) * P:(hb - 3) * P]
                if ob == 1:
                    return mt1[:, cps]
                if ob < 4:
                    return mt23[:, ob - 2, cps]
                if ob < 6:
                    return mt45[:, ob - 4, cps]
                return mt67[:, ob - 6, cps]

            def x_ap(lb, hb):
                if lb == 0:
                    return xp0[hb // 2][:, hb % 2, :]
                if lb == 1:
                    return x1h[hb // 4][:, hb % 4, :]
                return xl[lb][:, hb, :]

            sp = [rp.tile([P, 1], dt.float32, name=f"sp{i}", tag=f"sp{i}")
                  for i in range(NL)]

            # d-matmul bookkeeping: delay each block's last rowsum-MM into the
            # next MM group so the PE never waits on the DVE prod chain.
            state = {"pending": None}

            def flush_pending():
                if state["pending"] is None:
                    return
                pd_t, prod_t, lb = state["pending"]
                state["pending"] = None
                nc.tensor.matmul(pd_t[:], ones[:], prod_t[:],
                                 start=True, stop=True)
                # t = sigmoid(-d - c0 + ln(L-1)); r = t/(L-1)
                # accum_out gives S-partial = sum_l t over this block
                ls = slice(lb * LB, (lb + 1) * LB)
                nc.scalar.activation(t_s[:, ls], pd_t[:], AF.Sigmoid,
                                     bias=c0b[:, 0:1], scale=-1.0,
                                     accum_out=sp[lb][:])

            # ================= phase 1: y' ; d ; r ==================
            for lb in range(NL):
                pd = dp.tile([P, LB], dt.float32)
                acc = None
                for ob in range(NH):
                    py = yp.tile([P, LB], dt.float32, tag="mm")
                    for hb in range(NH):
                        nc.tensor.matmul(
                            py[:], mt_ap(ob, hb), x_ap(lb, hb),
                            start=(hb == 0), stop=(hb == NH - 1))
                    if ob == 1:
                        flush_pending()
                    # prod = (psum + u[ob]) * x   -- one fused DVE op
                    prod = wp.tile([P, LB], dt.float32r, tag="prod")
                    nc.vector.scalar_tensor_tensor(
                        prod[:], py[:], ub[:, ob:ob + 1], x_ap(lb, ob),
                        op0=ALU.add, op1=ALU.mult)
                    if acc is None:
                        acc = prod
                    else:
                        # last add emits bf16 so the rowsum MM stays in the
                        # bf16 pipeline (no fp32<->bf16 PE mode switch)
                        last = (ob == NH - 1)
                        # the lb0/ob4 add runs on gpsimd: the late DMAs
                        # queued behind it then start past the cold window
                        eng = nc.gpsimd if (lb == 0 and ob == 4) else nc.vector
                        nacc = wp.tile(
                            [P, LB],
                            dt.bfloat16 if last else dt.float32r,
                            name="naccb" if last else "nacc",
                            tag="paccb" if last else "pacc")
                        eng.tensor_tensor(nacc[:], acc[:], prod[:],
                                          op=ALU.add)
                        acc = nacc
                        if lb == 0 and ob == 4:
                            nc.gpsimd.dma_start(
                                xl[2][:], xT3[:, :, 2 * LB:3 * LB])
                            nc.gpsimd.dma_start(ones[:], ones_d[:])
                            nc.gpsimd.dma_start(
                                xl[3][:], xT3[:, :, 3 * LB:4 * LB])
                            nc.gpsimd.dma_start(wct8[:], WcT_d[:])
                state["pending"] = (pd, acc, lb)

            # ================= phase 2: z ; out ==================
            for lb in range(NL):
                for ob in range(NH):
                    last_grp = (lb == NL - 1 and ob == NH - 1)
                    nmm = 2 if last_grp else 1
                    mw = LB // nmm
                    pzs = []
                    for ck in range(nmm):
                        pz = yp.tile([P, mw], dt.float32, tag="mm")
                        for hb in range(NH):
                            nc.tensor.matmul(
                                pz[:], wct8[:, ob, hb * P:(hb + 1) * P],
                                x_ap(lb, hb)[:, ck * mw:(ck + 1) * mw],
                                start=(hb == 0), stop=(hb == NH - 1))
                        pzs.append(pz)
                    if lb == 0 and ob == 0:
                        flush_pending()   # last d-MM + sigmoid of phase 1
                        # colsum = (1 + sum(t)/(L-1)) - (L/(L-1))*t
                        s01 = rp.tile([P, 1], dt.float32)
                        nc.vector.tensor_tensor(s01[:], sp[0][:], sp[1][:],
                                                op=ALU.add)
                        s23 = rp.tile([P, 1], dt.float32)
                        nc.vector.tensor_tensor(s23[:], sp[2][:], sp[3][:],
                                                op=ALU.add)
                        s_all = rp.tile([P, 1], dt.float32)
                        nc.vector.tensor_tensor(s_all[:], s01[:], s23[:],
                                                op=ALU.add)
                        S1_t = rp.tile([P, 1], dt.float32)
                        nc.vector.tensor_scalar(
                            S1_t[:], s_all[:], 1.0 / (L - 1), 1.0,
                            op0=ALU.mult, op1=ALU.add)
                        nc.vector.tensor_scalar(
                            cs[:], t_s[:], -float(L) / (L - 1), S1_t[:],
                            op0=ALU.mult, op1=ALU.add)
                    for ck in range(nmm):
                        lo = lb * LB + ck * mw
                        lsc = slice(lo, lo + mw)
                        # z*colsum = (psum + bc[ob]) * cs  -- fused DVE op
                        zc = wp.tile([P, mw], dt.float32, tag="zc")
                        nc.vector.scalar_tensor_tensor(
                            zc[:], pzs[ck][:], bcb[:, ob:ob + 1], cs[:, lsc],
                            op0=ALU.add, op1=ALU.mult)
                        ot = wp.tile([P, mw], dt.float32, tag="ot")
                        nc.vector.tensor_scalar_add(
                            ot[:], zc[:], bob[:, ob:ob + 1])
                        if lb == NL - 1:
                            eng = nc.gpsimd
                        else:
                            eng = nc.scalar if (ob + ck) % 2 else nc.sync
                        eng.dma_start(out_d[ob * P:(ob + 1) * P, lsc], ot[:])

    nc.compile()
    return nc


def _get_nc():
    if "nc" not in _CACHE:
        _CACHE["nc"] = _build()
    return _CACHE["nc"]


def _prep_inputs(x, Wq, bq, Wk, bk, Wv, bv, Wo, bo):
    """Host-side precompute (fp64 for the fused weights) + per-core sharding."""
    f8 = np.float64
    bf = ml_dtypes.bfloat16
    M = (Wq.astype(f8).T @ Wk.astype(f8)).astype(np.float32)
    u = (Wk.astype(f8).T @ bq.astype(f8)
         + Wq.astype(f8).T @ bk.astype(f8)).astype(np.float32)
    c0 = np.float32(bq.astype(f8) @ bk.astype(f8))
    Wc = (Wo.astype(f8) @ Wv.astype(f8)).astype(np.float32)
    bc = (Wo.astype(f8) @ bv.astype(f8)).astype(np.float32)

    def _pack(WT):  # [H,H] (hin, hout) -> [P(hin%P), NH(ob), NH*P]
        t = WT.reshape(NH, P, NH, P)          # [hb, p, ob, c]
        return np.ascontiguousarray(
            t.transpose(1, 2, 0, 3).reshape(P, NH, NH * P).astype(bf))

    MT = _pack(M.T)
    WcT = _pack(Wc.T)
    ub = u.reshape(NH, P).T
    bcb = bc.reshape(NH, P).T
    bob = bo.astype(np.float32).reshape(NH, P).T
    c0b = np.full((P, 1), np.log(L - 1.0) - np.float64(c0), np.float32)
    cpack = np.ascontiguousarray(
        np.concatenate([ub, c0b, bcb, bob], axis=1).astype(np.float32))
    ones = np.ones((P, P), bf)

    shared = dict(MT=MT, WcT=WcT, cpack=cpack, ones=ones)
    in_maps = []
    for n in range(N_CORES):
        xT = np.ascontiguousarray(x[n].T.astype(bf))
        in_maps.append(dict(xT=xT, **shared))
    return in_maps


def kernel(x, Wq, bq, Wk, bk, Wv, bv, Wo, bo, _trace=False, _trace_kwargs=None):
    x, Wq, bq, Wk, bk, Wv, bv, Wo, bo = (
        np.asarray(a) for a in (x, Wq, bq, Wk, bk, Wv, bv, Wo, bo))
    nc = _get_nc()
    in_maps = _prep_inputs(x, Wq, bq, Wk, bk, Wv, bv, Wo, bo)
    res = run_bass_kernel_spmd(nc, in_maps, list(range(N_CORES)),
                               trace=_trace, **(_trace_kwargs or {}))
    out = np.empty((N, L, H), np.float32)
    for n in range(N_CORES):
        out[n] = res.results[n]["outT"].T
    if _trace:
        kernel.last_result = res
    return out
